# revision 22
# baseline (speedup 1.0000x reference)
"""Trainium2 Bass kernel: 16-head attention with RoPE (dense_transformer).

Sharding: tensor-parallel over heads. 8 cores x 2 heads each.
Each core: Wq/Wk/Wv column slice [1024,128], Wo row slice [128,1024],
full input; computes its heads' attention + partial output projection.
Host sums the 8 partial outputs (row-parallel Wo reduction) and adds bo.

Device layout is "transposed": Q^T/K^T/ctx^T are kept as [dim, seq] with
the head dim on SBUF partitions, so Q^T = Wq^T @ X^T comes straight out
of the PE, scores^T = K^T.T @ Q^T needs no transposes, and the softmax
denominator falls out of an extra ones-column appended to V.

v3: fully software-pipelined emission. The softmax exp runs only on the
scalar engine (133us total) while PE streams total ~165us, so the PE
instruction stream is emitted as: [scores matmul pair for unit (b,sh,h),
tile tt] followed by ~550ns of "filler" matmuls pulled from a FIFO
(second-batch QKV projection, context matmuls of the previous unit,
output-projection tiles). This keeps the exp engine continuously fed
while the PE never head-of-line blocks on PSUM slots waiting for exp.
DMA: host-relaid-out inputs with >=2KB contiguous per-partition lines,
split across both HWDGE queues.
"""

import sys

if "/opt/trn_rl_repo" not in sys.path:
    sys.path.insert(0, "/opt/trn_rl_repo")

import collections

import numpy as np
import ml_dtypes

B = 2
S = 2048
NS = B * S  # 4096
D = 1024
H = 16
DK = 64
NCORES = 8
HPC = H // NCORES  # heads per core = 2
DPC = HPC * DK  # model dims per core = 128

_cache = {}


def _build_nc(debug_taps=False):
    import concourse.bass as bass
    import concourse.tile as tile
    import concourse.mybir as mybir
    from concourse import bacc

    fp32 = mybir.dt.float32
    bf16 = mybir.dt.bfloat16
    Exp = mybir.ActivationFunctionType.Exp

    nc = bacc.Bacc("TRN2", debug=False, num_devices=NCORES)
    dbg = {}
    if debug_taps:
        for nm, shape in (
            ("dbg_qrot", [128, NS]),
            ("dbg_krot", [128, NS]),
            ("dbg_v", [128, 32 * 2 * (DK + 1)]),
            ("dbg_ctxT", [128, NS]),
        ):
            dbg[nm] = nc.dram_tensor(nm, shape, bf16, kind="ExternalOutput").ap()

    # xt: [p, st, ch, j] = X[st*512+j, ch*128+p]; per-partition line for a
    # given st is 8KB contiguous.
    xt = nc.dram_tensor("xt", [128, 8, 8, 512], bf16, kind="ExternalInput").ap()
    wq = nc.dram_tensor("wq", [128, 8, 128], bf16, kind="ExternalInput").ap()
    wk = nc.dram_tensor("wk", [128, 8, 128], bf16, kind="ExternalInput").ap()
    wv = nc.dram_tensor("wv", [128, 8, 128], bf16, kind="ExternalInput").ap()
    wo = nc.dram_tensor("wo", [DPC, D], bf16, kind="ExternalInput").ap()
    bq = nc.dram_tensor("bq", [DPC, 1], fp32, kind="ExternalInput").ap()
    bk = nc.dram_tensor("bk", [DPC, 1], fp32, kind="ExternalInput").ap()
    bv = nc.dram_tensor("bv", [1, DPC], bf16, kind="ExternalInput").ap()
    cos_d = nc.dram_tensor("cos32", [32, S], bf16, kind="ExternalInput").ap()
    sin_d = nc.dram_tensor("sin64", [64, S], bf16, kind="ExternalInput").ap()
    out_d = nc.dram_tensor("out", [D, NS], bf16, kind="ExternalOutput").ap()

    with tile.TileContext(nc) as tc:
        with (
            tc.tile_pool(name="persist", bufs=1) as persist,
            tc.tile_pool(name="ropetmp", bufs=1) as ropetmp,
            tc.tile_pool(name="att_sb", bufs=2) as att_sb,
            tc.tile_pool(name="op_sb", bufs=2) as op_sb,
            tc.tile_pool(name="den_p", bufs=2) as den_p,
            tc.tile_pool(name="r_sb", bufs=1) as r_sb,
            tc.tile_pool(name="r_dram", bufs=2, space="DRAM") as r_dram,
            tc.tile_pool(name="sc_ps", bufs=2, space="PSUM") as sc_ps,
            tc.tile_pool(name="ctx_ps", bufs=2, space="PSUM") as ctx_ps,
            tc.tile_pool(name="mix_ps", bufs=2, space="PSUM") as mix_ps,
        ):
            qrot = persist.tile([128, NS], bf16, tag="qrot")
            krot = persist.tile([128, NS], bf16, tag="krot")
            v_sb = persist.tile([128, 32, 2 * (DK + 1)], bf16, tag="v")
            ctxT = persist.tile([128, NS], bf16, tag="ctxT")
            wo_sb = persist.tile([128, 8, 128], bf16, tag="wo")
            cos_sb = persist.tile([128, S], bf16, tag="cos")
            sin_sb = persist.tile([128, S], bf16, tag="sin")
            xt_sb = persist.tile([128, 8, 8, 512], bf16, tag="xt")
            wq_sb = persist.tile([128, 8, 128], bf16, tag="wq")
            wk_sb = persist.tile([128, 8, 128], bf16, tag="wk")
            wv_sb = persist.tile([128, 8, 128], bf16, tag="wv")
            bq_sb = persist.tile([128, 1], fp32, tag="bq")
            bk_sb = persist.tile([128, 1], fp32, tag="bk")
            bvb = persist.tile([128, 128], bf16, tag="bvb")
            dn128 = den_p.tile([128, 64], fp32, tag="dn128", bufs=1)
            recip128 = den_p.tile([128, 64], bf16, tag="recip", bufs=1)

            # ---- input DMA schedule ----
            nc.sync.dma_start(wq_sb[:], wq)
            nc.scalar.dma_start(wk_sb[:], wk)
            nc.scalar.dma_start(wv_sb[:], wv)
            for st in (0, 2, 4, 6):
                nc.sync.dma_start(xt_sb[:, st], xt[:, st])
            nc.scalar.dma_start(cos_sb[0:32, :], cos_d)
            # (tables follow wv; b0 odd chunks follow tables on scalar)
            nc.scalar.dma_start(sin_sb[0:64, :], sin_d)
            nc.scalar.dma_start(cos_sb[32:64, :], cos_sb[0:32, :])
            nc.scalar.dma_start(cos_sb[64:128, :], cos_sb[0:64, :])
            nc.scalar.dma_start(sin_sb[64:128, :], sin_sb[0:64, :])
            nc.gpsimd.dma_start(bq_sb[:], bq)
            nc.gpsimd.dma_start(bk_sb[:], bk)
            nc.gpsimd.dma_start(bvb[:], bv.to_broadcast((128, 128)))
            nc.scalar.dma_start(xt_sb[:, 1], xt[:, 1])
            nc.scalar.dma_start(xt_sb[:, 3], xt[:, 3])
            nc.gpsimd.dma_start(xt_sb[:, 5], xt[:, 5])
            nc.gpsimd.dma_start(xt_sb[:, 7], xt[:, 7])
            nc.sync.dma_start(wo_sb[:], wo.rearrange("p (j m) -> p j m", m=128))

            # ---------------- emission helpers ----------------
            def qk_group(w_sb_, b_sb_, st, rot, drain_eng):
                def fn():
                    ps = mix_ps.tile([128, 512], fp32, tag="mix")
                    for ch in range(8):
                        nc.tensor.matmul(
                            ps[:], w_sb_[:, ch, :], xt_sb[:, st, ch, :],
                            start=(ch == 0), stop=(ch == 7),
                        )
                    sl = slice(st * 512, (st + 1) * 512)
                    if drain_eng == "scalar":
                        nc.scalar.add(rot[:, sl], ps[:], add=b_sb_[:])
                    else:
                        nc.vector.tensor_scalar_add(rot[:, sl], ps[:], b_sb_[:])
                return fn

            def v_group(tt, drain_eng="vector"):
                def fn():
                    st, tv = tt // 4, tt % 4
                    ps = mix_ps.tile([128, 512], fp32, tag="mix")
                    psv = ps[:, 0:128]
                    for ch in range(8):
                        nc.tensor.matmul(
                            psv,
                            xt_sb[:, st, ch, tv * 128 : (tv + 1) * 128],
                            wv_sb[:, ch, :],
                            start=(ch == 0), stop=(ch == 7),
                        )
                    dst = v_sb[:, tt].rearrange("p (h x) -> p h x", h=2)[:, :, 0:DK]
                    if drain_eng == "scalar":
                        nc.scalar.add(dst, psv, add=0.0)
                        nc.gpsimd.tensor_add(dst, dst, bvb[:])
                    else:
                        nc.vector.tensor_add(dst, psv, bvb[:])
                return fn

            def rope_st(st):
                def fn():
                    csl = slice(st * 512, (st + 1) * 512)
                    cs = slice((st % 4) * 512, (st % 4 + 1) * 512)
                    for rot in (krot, qrot):
                        swap = ropetmp.tile(
                            [128, 512], bf16, tag="swap", bufs=4, name="swap"
                        )
                        for g in (0, 64):
                            nc.gpsimd.dma_start(
                                swap[g : g + 32, :], rot[g + 32 : g + 64, csl]
                            )
                            nc.gpsimd.dma_start(
                                swap[g + 32 : g + 64, :], rot[g : g + 32, csl]
                            )
                        nc.vector.tensor_mul(swap[:], swap[:], sin_sb[:, cs])
                        nc.vector.tensor_mul(rot[:, csl], rot[:, csl], cos_sb[:, cs])
                        nc.vector.tensor_add(rot[:, csl], rot[:, csl], swap[:])
                return fn

            # ctx state: pc tiles keyed by (b, sh, h, sq); den rows per (b, sh)
            pc_tiles = {}
            den_rows = {}

            def ctx_chunk(b, sh, h, sq, t0, expS):
                def fn():
                    key = (b, sh, h, sq)
                    if t0 == 0:
                        pc_tiles[key] = ctx_ps.tile([DK + 1, 512], fp32, tag="pc", name="pc")
                    pc = pc_tiles[key]
                    for tt in range(t0, t0 + 4):
                        nc.tensor.matmul(
                            pc[:],
                            v_sb[:, b * 16 + tt, h * (DK + 1) : (h + 1) * (DK + 1)],
                            expS[:, tt, sq * 512 : (sq + 1) * 512],
                            start=(tt == 0), stop=(tt == 15),
                        )
                    if t0 == 12:
                        # drain: ctx rows -> ctxT (pre-division), den row
                        st_i = sh * 2 + sq
                        ds0 = b * S + st_i * 512
                        if (b, sh) not in den_rows:
                            den_rows[(b, sh)] = den_p.tile(
                                [1, 2048], fp32, tag="den", name="den_row"
                            )
                        doff = (h * 2 + sq) * 512
                        nc.vector.tensor_copy(
                            den_rows[(b, sh)][:, doff : doff + 512],
                            pc[DK : DK + 1, :],
                        )
                        if h == 0:
                            nc.vector.tensor_copy(
                                ctxT[0:DK, ds0 : ds0 + 512], pc[0:DK, :]
                            )
                        else:
                            stg = r_sb.tile([64, 512], bf16, tag="stg", bufs=1, name="stg")
                            nc.vector.tensor_copy(stg[:], pc[0:DK, :])
                            nc.vector.stream_shuffle(
                                ctxT[DK : 2 * DK, ds0 : ds0 + 512],
                                stg[:],
                                mask=list(range(32)),
                            )
                        del pc_tiles[key]
                return fn

            def den_div(b, sh):
                def fn():
                    # denominators for this (b, sh): 4 rows of 512
                    coff = (b * 2 + sh) * 16
                    dd = r_dram.tile([2048], fp32, tag="den_dram")
                    rd = r_dram.tile([2048], bf16, tag="rec_dram")
                    nc.sync.dma_start(
                        dd.rearrange("(o c) -> o c", o=1),
                        den_rows.pop((b, sh))[0:1, :],
                    )
                    nc.sync.dma_start(
                        dn128[:, coff : coff + 16],
                        dd.rearrange("(p c) -> p c", p=128),
                    )
                    with nc.allow_low_precision(
                        reason="bf16 softmax reciprocal is within kernel tolerance"
                    ):
                        nc.vector.reciprocal(
                            recip128[:, coff : coff + 16],
                            dn128[:, coff : coff + 16],
                        )
                    nc.sync.dma_start(
                        rd.rearrange("(p c) -> p c", p=128),
                        recip128[:, coff : coff + 16],
                    )
                    Rt = r_sb.tile([128, 1024], bf16, tag="R")
                    for h in range(HPC):
                        nc.sync.dma_start(
                            Rt[h * DK : (h + 1) * DK, :],
                            rd[h * 1024 : (h + 1) * 1024]
                            .rearrange("(o s) -> o s", o=1)
                            .to_broadcast((DK, 1024)),
                        )
                    c0 = b * S + sh * 1024
                    nc.vector.tensor_mul(
                        ctxT[:, c0 : c0 + 1024], ctxT[:, c0 : c0 + 1024], Rt[:]
                    )
                return fn

            def op_tile(b, stp, oc, tail=False):
                def fn():
                    c0 = b * S + stp * 1024
                    ob2 = op_sb.tile([128, 1024], bf16, tag="ob2")
                    if tail and oc % 2 == 0:
                        ps2 = sc_ps.tile([128, 1024], fp32, tag="ps", name="ps2")
                        pa, pb = ps2[:, 0:512], ps2[:, 512:1024]
                    else:
                        pa = mix_ps.tile([128, 512], fp32, tag="mix", name="pa")
                        pb = mix_ps.tile([128, 512], fp32, tag="mix", name="pb")
                    nc.tensor.matmul(
                        pa, wo_sb[:, oc, :], ctxT[:, c0 : c0 + 512],
                        start=True, stop=True,
                    )
                    nc.tensor.matmul(
                        pb, wo_sb[:, oc, :], ctxT[:, c0 + 512 : c0 + 1024],
                        start=True, stop=True,
                    )
                    nc.vector.tensor_copy(ob2[:, 0:512], pa)
                    nc.scalar.copy(ob2[:, 512:1024], pb)
                    nc.sync.dma_start(
                        out_d[oc * 128 : (oc + 1) * 128, c0 : c0 + 1024], ob2[:]
                    )
                return fn

            # ---------------- filler queue + debt-carry metering ------------
            fq_hi = collections.deque()
            fq_lo = fq_hi  # single FIFO: emission order must respect data deps
            acc = [0.0]

            def pull(budget):
                acc[0] += budget
                while acc[0] > 0 and fq_hi:
                    cost, fn = fq_hi.popleft()
                    fn()
                    acc[0] -= cost

            # upfront (unmetered, overlaps the input DMA): st0/st1 K/Q/rope
            # plus V(b0) for those chunks slotted into the DMA-wait gaps
            for st in (0, 1):
                qk_group(wk_sb, bk_sb, st, krot, "vector")()
                qk_group(wq_sb, bq_sb, st, qrot, "vector")()
                rope_st(st)()
                for tv in range(4):
                    v_group(st * 4 + tv)()
            nc.vector.memset(
                v_sb[:].rearrange("p t (h x) -> p t h x", x=DK + 1)[:, 0:16, :, DK],
                1.0,
            )

            # seed fillers: st2/st3 K/Q/rope + rest of V(b0)
            for st in (2, 3):
                fq_lo.append((2000, qk_group(wk_sb, bk_sb, st, krot, "vector")))
                fq_lo.append((2000, qk_group(wq_sb, bq_sb, st, qrot, "vector")))
                fq_lo.append((0, rope_st(st)))
            for tt in range(8, 16):
                fq_lo.append((700, v_group(tt)))

            units = [
                (b, sh, h) for b in range(B) for sh in range(2) for h in range(HPC)
            ]
            for j, (b, sh, h) in enumerate(units):
                hh = h * DK
                expS = att_sb.tile([128, 16, 1024], bf16, tag="expS")
                for tt in range(16):
                    ps = sc_ps.tile([128, 1024], fp32, tag="ps")
                    for si in range(2):
                        s0 = b * S + sh * 1024 + si * 512
                        nc.tensor.matmul(
                            ps[:, si * 512 : (si + 1) * 512],
                            krot[
                                hh : hh + DK,
                                b * S + tt * 128 : b * S + (tt + 1) * 128,
                            ],
                            qrot[hh : hh + DK, s0 : s0 + 512],
                            start=True, stop=True,
                        )
                    nc.scalar.activation(expS[:, tt, :], ps[:], Exp, scale=0.125)
                    pull(960)

                # enqueue this unit's ctx work (latency-sensitive)
                for sq in range(2):
                    for t0 in (0, 4, 8, 12):
                        fq_hi.append((1100, ctx_chunk(b, sh, h, sq, t0, expS)))
                if h == 1:
                    fq_hi.append((0, den_div(b, sh)))
                    tail = j == len(units) - 1
                    for oc in range(8):
                        fq_hi.append((700, op_tile(b, sh, oc, tail=tail)))
                # strategic phase-1(b1) injection
                if j == 0:
                    for st in range(4, 8):
                        fq_lo.append((2000, qk_group(wk_sb, bk_sb, st, krot, "vector")))
                        fq_lo.append((2000, qk_group(wq_sb, bq_sb, st, qrot, "vector")))
                        fq_lo.append((0, rope_st(st)))
                if j == 1:
                    for tt in range(16, 32):
                        fq_lo.append((700, v_group(tt)))
                    fq_lo.append(
                        (
                            200,
                            lambda: nc.vector.memset(
                                v_sb[:].rearrange(
                                    "p t (h x) -> p t h x", x=DK + 1
                                )[:, 16:32, :, DK],
                                1.0,
                            ),
                        )
                    )

            pull(float("inf"))

            if debug_taps:
                nc.sync.dma_start(dbg["dbg_qrot"], qrot[:])
                nc.sync.dma_start(dbg["dbg_krot"], krot[:])
                nc.sync.dma_start(dbg["dbg_v"], v_sb[:].rearrange("p a b -> p (a b)"))
                nc.sync.dma_start(dbg["dbg_ctxT"], ctxT[:])

    nc.compile()
    return nc


def _rope_tables():
    pos = np.arange(S, dtype=np.float64)
    inv_freq = np.exp(np.arange(0, DK, 2, dtype=np.float64) * (-np.log(10000.0) / DK))
    ang = pos[:, None] * inv_freq[None, :]  # [S, 32]
    c = np.cos(ang).astype(np.float32).T  # [32, S]
    s = np.sin(ang).astype(np.float32).T
    cos32 = c
    sin64 = np.concatenate([-s, s], axis=0)  # [64, S]
    return cos32, sin64


def _prep_inputs(inputs, Wq, bq, Wk, bk, Wv, bv, Wo):
    bf = ml_dtypes.bfloat16
    x2 = np.asarray(inputs, dtype=np.float32).reshape(NS, D)
    # [p, st, ch, j] = X[st*512+j, ch*128+p]
    xt = np.ascontiguousarray(
        x2.reshape(8, 512, 8, 128).transpose(3, 0, 2, 1)
    ).astype(bf)
    cos32, sin64 = _rope_tables()
    cos_b = cos32.astype(bf)
    sin_b = sin64.astype(bf)

    def wprep(W, sl):
        return np.ascontiguousarray(
            W[:, sl].reshape(8, 128, DPC).transpose(1, 0, 2)
        ).astype(bf)

    in_maps = []
    for c in range(NCORES):
        sl = slice(c * DPC, (c + 1) * DPC)
        in_maps.append(
            {
                "xt": xt,
                "wq": wprep(Wq, sl),
                "wk": wprep(Wk, sl),
                "wv": wprep(Wv, sl),
                "wo": np.ascontiguousarray(Wo[sl, :]).astype(bf),
                "bq": np.ascontiguousarray(bq[sl]).reshape(DPC, 1).astype(np.float32),
                "bk": np.ascontiguousarray(bk[sl]).reshape(DPC, 1).astype(np.float32),
                "bv": np.ascontiguousarray(bv[sl]).reshape(1, DPC).astype(bf),
                "cos32": cos_b,
                "sin64": sin_b,
            }
        )
    return in_maps


def _get_nc():
    if "nc" not in _cache:
        _cache["nc"] = _build_nc()
    return _cache["nc"]


def run(inputs_dict, trace=False):
    """Build (cached), run on 8 cores, assemble full output. Returns
    (output fp32 [B,S,D], BassKernelResults)."""
    from concourse.bass_utils import run_bass_kernel_spmd

    nc = _get_nc()
    in_maps = _prep_inputs(
        inputs_dict["inputs"],
        inputs_dict["Wq"],
        inputs_dict["bq"],
        inputs_dict["Wk"],
        inputs_dict["bk"],
        inputs_dict["Wv"],
        inputs_dict["bv"],
        inputs_dict["Wo"],
    )
    res = run_bass_kernel_spmd(
        nc, in_maps, core_ids=list(range(NCORES)), trace=trace
    )
    acc = np.zeros((D, NS), dtype=np.float32)
    for r in res.results:
        acc += r["out"].astype(np.float32)
    out = acc.T.reshape(B, S, D) + np.asarray(inputs_dict["bo"], dtype=np.float32)
    return out.astype(np.float32), res


def kernel(**inputs):
    out, _ = run(inputs, trace=False)
    return out


# revision 23
# speedup vs baseline: 1.1568x; 1.1568x over previous
"""Trainium2 Bass kernel: 16-head attention with RoPE (dense_transformer).

Sharding: tensor-parallel over heads. 8 cores x 2 heads each.
Each core: Wq/Wk/Wv column slice [1024,128], Wo row slice [128,1024],
full input; computes its heads' attention + partial output projection.
Host sums the 8 partial outputs (row-parallel Wo reduction) and adds bo.

Device layout is "transposed": Q^T/K^T/ctx^T are kept as [dim, seq] with
the head dim on SBUF partitions, so Q^T = Wq^T @ X^T comes straight out
of the PE, scores^T = K^T.T @ Q^T needs no transposes, and the softmax
denominator falls out of an extra ones-column appended to V.

v3: fully software-pipelined emission. The softmax exp runs only on the
scalar engine (133us total) while PE streams total ~165us, so the PE
instruction stream is emitted as: [scores matmul pair for unit (b,sh,h),
tile tt] followed by ~550ns of "filler" matmuls pulled from a FIFO
(second-batch QKV projection, context matmuls of the previous unit,
output-projection tiles). This keeps the exp engine continuously fed
while the PE never head-of-line blocks on PSUM slots waiting for exp.
DMA: host-relaid-out inputs with >=2KB contiguous per-partition lines,
split across both HWDGE queues.
"""

import sys

if "/opt/trn_rl_repo" not in sys.path:
    sys.path.insert(0, "/opt/trn_rl_repo")

import collections

import numpy as np
import ml_dtypes

B = 2
S = 2048
NS = B * S  # 4096
D = 1024
H = 16
DK = 64
NCORES = 8
HPC = H // NCORES  # heads per core = 2
DPC = HPC * DK  # model dims per core = 128

_cache = {}


def _build_nc(debug_taps=False):
    import concourse.bass as bass
    import concourse.tile as tile
    import concourse.mybir as mybir
    from concourse import bacc

    fp32 = mybir.dt.float32
    bf16 = mybir.dt.bfloat16
    Exp = mybir.ActivationFunctionType.Exp

    nc = bacc.Bacc("TRN2", debug=False, num_devices=NCORES)
    dbg = {}
    if debug_taps:
        for nm, shape in (
            ("dbg_qrot", [128, NS]),
            ("dbg_krot", [128, NS]),
            ("dbg_v", [128, 32 * 2 * (DK + 1)]),
            ("dbg_ctxT", [128, NS]),
        ):
            dbg[nm] = nc.dram_tensor(nm, shape, bf16, kind="ExternalOutput").ap()

    # xt: [p, st, ch, j] = X[st*512+j, ch*128+p]; per-partition line for a
    # given st is 8KB contiguous.
    xt = nc.dram_tensor("xt", [128, 8, 8, 512], bf16, kind="ExternalInput").ap()
    wq = nc.dram_tensor("wq", [128, 8, 128], bf16, kind="ExternalInput").ap()
    wk = nc.dram_tensor("wk", [128, 8, 128], bf16, kind="ExternalInput").ap()
    wv = nc.dram_tensor("wv", [128, 8, 128], bf16, kind="ExternalInput").ap()
    wo = nc.dram_tensor("wo", [DPC, D], bf16, kind="ExternalInput").ap()
    bq = nc.dram_tensor("bq", [DPC, 1], fp32, kind="ExternalInput").ap()
    bk = nc.dram_tensor("bk", [DPC, 1], fp32, kind="ExternalInput").ap()
    bv = nc.dram_tensor("bv", [1, DPC], bf16, kind="ExternalInput").ap()
    cos_d = nc.dram_tensor("cos32", [32, S], bf16, kind="ExternalInput").ap()
    sin_d = nc.dram_tensor("sin64", [64, S], bf16, kind="ExternalInput").ap()
    out_d = nc.dram_tensor("out", [D, NS], bf16, kind="ExternalOutput").ap()

    with tile.TileContext(nc) as tc:
        with (
            tc.tile_pool(name="persist", bufs=1) as persist,
            tc.tile_pool(name="ropetmp", bufs=1) as ropetmp,
            tc.tile_pool(name="att_sb", bufs=2) as att_sb,
            tc.tile_pool(name="op_sb", bufs=2) as op_sb,
            tc.tile_pool(name="den_p", bufs=2) as den_p,
            tc.tile_pool(name="r_sb", bufs=1) as r_sb,
            tc.tile_pool(name="r_dram", bufs=2, space="DRAM") as r_dram,
            tc.tile_pool(name="sc_ps", bufs=2, space="PSUM") as sc_ps,
            tc.tile_pool(name="ctx_ps", bufs=2, space="PSUM") as ctx_ps,
            tc.tile_pool(name="mix_ps", bufs=2, space="PSUM") as mix_ps,
        ):
            qrot = persist.tile([128, NS], bf16, tag="qrot")
            krot = persist.tile([128, NS], bf16, tag="krot")
            v_sb = persist.tile([128, 32, 2 * (DK + 1)], bf16, tag="v")
            ctxT = persist.tile([128, NS], bf16, tag="ctxT")
            wo_sb = persist.tile([128, 8, 128], bf16, tag="wo")
            cos_sb = persist.tile([128, S], bf16, tag="cos")
            sin_sb = persist.tile([128, S], bf16, tag="sin")
            xt_sb = persist.tile([128, 8, 8, 512], bf16, tag="xt")
            wq_sb = persist.tile([128, 8, 128], bf16, tag="wq")
            wk_sb = persist.tile([128, 8, 128], bf16, tag="wk")
            wv_sb = persist.tile([128, 8, 128], bf16, tag="wv")
            bq_sb = persist.tile([128, 1], fp32, tag="bq")
            bk_sb = persist.tile([128, 1], fp32, tag="bk")
            bvb = persist.tile([128, 128], bf16, tag="bvb")
            dn128 = den_p.tile([128, 64], fp32, tag="dn128", bufs=1)
            recip128 = den_p.tile([128, 64], bf16, tag="recip", bufs=1)

            # ---- input DMA schedule ----
            nc.sync.dma_start(wq_sb[:], wq)
            nc.scalar.dma_start(wk_sb[:], wk)
            nc.scalar.dma_start(wv_sb[:], wv)
            for st in (0, 2, 4, 6):
                nc.sync.dma_start(xt_sb[:, st], xt[:, st])
            nc.scalar.dma_start(cos_sb[0:32, :], cos_d)
            nc.scalar.dma_start(sin_sb[0:64, :], sin_d)
            nc.gpsimd.dma_start(cos_sb[32:64, :], cos_sb[0:32, :])
            nc.gpsimd.dma_start(cos_sb[64:128, :], cos_sb[0:64, :])
            nc.gpsimd.dma_start(sin_sb[64:128, :], sin_sb[0:64, :])
            nc.gpsimd.dma_start(bq_sb[:], bq)
            nc.gpsimd.dma_start(bk_sb[:], bk)
            nc.gpsimd.dma_start(bvb[:], bv.to_broadcast((128, 128)))
            nc.scalar.dma_start(xt_sb[:, 1], xt[:, 1])
            nc.scalar.dma_start(xt_sb[:, 3], xt[:, 3])
            nc.gpsimd.dma_start(xt_sb[:, 5], xt[:, 5])
            nc.gpsimd.dma_start(xt_sb[:, 7], xt[:, 7])
            nc.sync.dma_start(wo_sb[:], wo.rearrange("p (j m) -> p j m", m=128))

            # ---------------- emission helpers ----------------
            def qk_group(w_sb_, b_sb_, st, rot, drain_eng):
                def fn():
                    ps = mix_ps.tile([128, 512], fp32, tag="mix")
                    for ch in range(8):
                        nc.tensor.matmul(
                            ps[:], w_sb_[:, ch, :], xt_sb[:, st, ch, :],
                            start=(ch == 0), stop=(ch == 7),
                        )
                    sl = slice(st * 512, (st + 1) * 512)
                    if drain_eng == "scalar":
                        nc.scalar.add(rot[:, sl], ps[:], add=b_sb_[:])
                    else:
                        nc.vector.tensor_scalar_add(rot[:, sl], ps[:], b_sb_[:])
                return fn

            def v_group(tt, drain_eng="vector"):
                def fn():
                    st, tv = tt // 4, tt % 4
                    ps = mix_ps.tile([128, 512], fp32, tag="mix")
                    psv = ps[:, 0:128]
                    for ch in range(8):
                        nc.tensor.matmul(
                            psv,
                            xt_sb[:, st, ch, tv * 128 : (tv + 1) * 128],
                            wv_sb[:, ch, :],
                            start=(ch == 0), stop=(ch == 7),
                        )
                    dst = v_sb[:, tt].rearrange("p (h x) -> p h x", h=2)[:, :, 0:DK]
                    if drain_eng == "scalar":
                        nc.scalar.add(dst, psv, add=0.0)
                        nc.gpsimd.tensor_add(dst, dst, bvb[:])
                    else:
                        nc.vector.tensor_add(dst, psv, bvb[:])
                return fn

            def rope_st(st):
                def fn():
                    csl = slice(st * 512, (st + 1) * 512)
                    cs = slice((st % 4) * 512, (st % 4 + 1) * 512)
                    for rot in (krot, qrot):
                        swap = ropetmp.tile(
                            [128, 512], bf16, tag="swap", bufs=4, name="swap"
                        )
                        for g in (0, 64):
                            nc.gpsimd.dma_start(
                                swap[g : g + 32, :], rot[g + 32 : g + 64, csl]
                            )
                            nc.gpsimd.dma_start(
                                swap[g + 32 : g + 64, :], rot[g : g + 32, csl]
                            )
                        nc.vector.tensor_mul(swap[:], swap[:], sin_sb[:, cs])
                        nc.vector.tensor_mul(rot[:, csl], rot[:, csl], cos_sb[:, cs])
                        nc.vector.tensor_add(rot[:, csl], rot[:, csl], swap[:])
                return fn

            # ctx state: pc tiles keyed by (b, sh, h, sq); den rows per (b, sh)
            pc_tiles = {}
            den_rows = {}

            def ctx_chunk(b, sh, h, sq, t0, expS):
                def fn():
                    key = (b, sh, h, sq)
                    if t0 == 0:
                        pc_tiles[key] = ctx_ps.tile([DK + 1, 512], fp32, tag="pc", name="pc")
                    pc = pc_tiles[key]
                    for tt in range(t0, t0 + 4):
                        nc.tensor.matmul(
                            pc[:],
                            v_sb[:, b * 16 + tt, h * (DK + 1) : (h + 1) * (DK + 1)],
                            expS[:, tt, sq * 512 : (sq + 1) * 512],
                            start=(tt == 0), stop=(tt == 15),
                        )
                    if t0 == 12:
                        # drain: ctx rows -> ctxT (pre-division), den row
                        st_i = sh * 2 + sq
                        ds0 = b * S + st_i * 512
                        if (b, sh) not in den_rows:
                            den_rows[(b, sh)] = den_p.tile(
                                [1, 2048], fp32, tag="den", name="den_row"
                            )
                        doff = (h * 2 + sq) * 512
                        nc.vector.tensor_copy(
                            den_rows[(b, sh)][:, doff : doff + 512],
                            pc[DK : DK + 1, :],
                        )
                        if h == 0:
                            nc.vector.tensor_copy(
                                ctxT[0:DK, ds0 : ds0 + 512], pc[0:DK, :]
                            )
                        else:
                            stg = r_sb.tile([64, 512], bf16, tag="stg", bufs=1, name="stg")
                            nc.vector.tensor_copy(stg[:], pc[0:DK, :])
                            nc.vector.stream_shuffle(
                                ctxT[DK : 2 * DK, ds0 : ds0 + 512],
                                stg[:],
                                mask=list(range(32)),
                            )
                        del pc_tiles[key]
                return fn

            def den_div(b, sh):
                def fn():
                    # denominators for this (b, sh): 4 rows of 512
                    coff = (b * 2 + sh) * 16
                    dd = r_dram.tile([2048], fp32, tag="den_dram")
                    rd = r_dram.tile([2048], bf16, tag="rec_dram")
                    nc.sync.dma_start(
                        dd.rearrange("(o c) -> o c", o=1),
                        den_rows.pop((b, sh))[0:1, :],
                    )
                    nc.sync.dma_start(
                        dn128[:, coff : coff + 16],
                        dd.rearrange("(p c) -> p c", p=128),
                    )
                    with nc.allow_low_precision(
                        reason="bf16 softmax reciprocal is within kernel tolerance"
                    ):
                        nc.vector.reciprocal(
                            recip128[:, coff : coff + 16],
                            dn128[:, coff : coff + 16],
                        )
                    nc.sync.dma_start(
                        rd.rearrange("(p c) -> p c", p=128),
                        recip128[:, coff : coff + 16],
                    )
                    Rt = r_sb.tile([128, 1024], bf16, tag="R")
                    for h in range(HPC):
                        nc.sync.dma_start(
                            Rt[h * DK : (h + 1) * DK, :],
                            rd[h * 1024 : (h + 1) * 1024]
                            .rearrange("(o s) -> o s", o=1)
                            .to_broadcast((DK, 1024)),
                        )
                    c0 = b * S + sh * 1024
                    nc.vector.tensor_mul(
                        ctxT[:, c0 : c0 + 1024], ctxT[:, c0 : c0 + 1024], Rt[:]
                    )
                return fn

            def op_tile(b, stp, oc, tail=False):
                def fn():
                    c0 = b * S + stp * 1024
                    ob2 = op_sb.tile([128, 1024], bf16, tag="ob2")
                    if tail and oc % 2 == 0:
                        ps2 = sc_ps.tile([128, 1024], fp32, tag="ps", name="ps2")
                        pa, pb = ps2[:, 0:512], ps2[:, 512:1024]
                    else:
                        pa = mix_ps.tile([128, 512], fp32, tag="mix", name="pa")
                        pb = mix_ps.tile([128, 512], fp32, tag="mix", name="pb")
                    nc.tensor.matmul(
                        pa, wo_sb[:, oc, :], ctxT[:, c0 : c0 + 512],
                        start=True, stop=True,
                    )
                    nc.tensor.matmul(
                        pb, wo_sb[:, oc, :], ctxT[:, c0 + 512 : c0 + 1024],
                        start=True, stop=True,
                    )
                    nc.vector.tensor_copy(ob2[:, 0:512], pa)
                    nc.scalar.copy(ob2[:, 512:1024], pb)
                    nc.sync.dma_start(
                        out_d[oc * 128 : (oc + 1) * 128, c0 : c0 + 1024], ob2[:]
                    )
                return fn

            # ---------------- filler queue + debt-carry metering ------------
            fq_hi = collections.deque()
            fq_lo = fq_hi  # single FIFO: emission order must respect data deps
            acc = [0.0]

            def pull(budget):
                acc[0] += budget
                while acc[0] > 0 and fq_hi:
                    cost, fn = fq_hi.popleft()
                    fn()
                    acc[0] -= cost

            # upfront (unmetered, overlaps the input DMA): st0/st1 K/Q/rope
            # plus V(b0) for those chunks slotted into the DMA-wait gaps
            for st in (0, 1):
                qk_group(wk_sb, bk_sb, st, krot, "vector")()
                qk_group(wq_sb, bq_sb, st, qrot, "vector")()
                rope_st(st)()
                for tv in range(4):
                    v_group(st * 4 + tv)()
            nc.vector.memset(
                v_sb[:].rearrange("p t (h x) -> p t h x", x=DK + 1)[:, 0:16, :, DK],
                1.0,
            )

            # seed fillers: st2/st3 K/Q/rope + rest of V(b0)
            for st in (2, 3):
                fq_lo.append((2000, qk_group(wk_sb, bk_sb, st, krot, "vector")))
                fq_lo.append((2000, qk_group(wq_sb, bq_sb, st, qrot, "vector")))
                fq_lo.append((0, rope_st(st)))
            for tt in range(8, 16):
                fq_lo.append((700, v_group(tt)))

            units = [
                (b, sh, h) for b in range(B) for sh in range(2) for h in range(HPC)
            ]
            for j, (b, sh, h) in enumerate(units):
                hh = h * DK
                expS = att_sb.tile([128, 16, 1024], bf16, tag="expS")
                for tt in range(16):
                    ps = sc_ps.tile([128, 1024], fp32, tag="ps")
                    for si in range(2):
                        s0 = b * S + sh * 1024 + si * 512
                        nc.tensor.matmul(
                            ps[:, si * 512 : (si + 1) * 512],
                            krot[
                                hh : hh + DK,
                                b * S + tt * 128 : b * S + (tt + 1) * 128,
                            ],
                            qrot[hh : hh + DK, s0 : s0 + 512],
                            start=True, stop=True,
                        )
                    nc.scalar.activation(expS[:, tt, :], ps[:], Exp, scale=0.125)
                    pull(960)

                # enqueue this unit's ctx work (latency-sensitive)
                for sq in range(2):
                    for t0 in (0, 4, 8, 12):
                        fq_hi.append((1100, ctx_chunk(b, sh, h, sq, t0, expS)))
                if h == 1:
                    fq_hi.append((0, den_div(b, sh)))
                    tail = j == len(units) - 1
                    for oc in range(8):
                        fq_hi.append((700, op_tile(b, sh, oc, tail=tail)))
                # strategic phase-1(b1) injection
                if j == 0:
                    for st in range(4, 8):
                        fq_lo.append((2000, qk_group(wk_sb, bk_sb, st, krot, "vector")))
                        fq_lo.append((2000, qk_group(wq_sb, bq_sb, st, qrot, "vector")))
                        fq_lo.append((0, rope_st(st)))
                if j == 1:
                    for tt in range(16, 32):
                        fq_lo.append((700, v_group(tt)))
                    fq_lo.append(
                        (
                            200,
                            lambda: nc.vector.memset(
                                v_sb[:].rearrange(
                                    "p t (h x) -> p t h x", x=DK + 1
                                )[:, 16:32, :, DK],
                                1.0,
                            ),
                        )
                    )

            pull(float("inf"))

            if debug_taps:
                nc.sync.dma_start(dbg["dbg_qrot"], qrot[:])
                nc.sync.dma_start(dbg["dbg_krot"], krot[:])
                nc.sync.dma_start(dbg["dbg_v"], v_sb[:].rearrange("p a b -> p (a b)"))
                nc.sync.dma_start(dbg["dbg_ctxT"], ctxT[:])

    nc.compile()
    return nc


def _rope_tables():
    pos = np.arange(S, dtype=np.float64)
    inv_freq = np.exp(np.arange(0, DK, 2, dtype=np.float64) * (-np.log(10000.0) / DK))
    ang = pos[:, None] * inv_freq[None, :]  # [S, 32]
    c = np.cos(ang).astype(np.float32).T  # [32, S]
    s = np.sin(ang).astype(np.float32).T
    cos32 = c
    sin64 = np.concatenate([-s, s], axis=0)  # [64, S]
    return cos32, sin64


def _prep_inputs(inputs, Wq, bq, Wk, bk, Wv, bv, Wo):
    bf = ml_dtypes.bfloat16
    x2 = np.asarray(inputs, dtype=np.float32).reshape(NS, D)
    # [p, st, ch, j] = X[st*512+j, ch*128+p]
    xt = np.ascontiguousarray(
        x2.reshape(8, 512, 8, 128).transpose(3, 0, 2, 1)
    ).astype(bf)
    cos32, sin64 = _rope_tables()
    cos_b = cos32.astype(bf)
    sin_b = sin64.astype(bf)

    def wprep(W, sl):
        return np.ascontiguousarray(
            W[:, sl].reshape(8, 128, DPC).transpose(1, 0, 2)
        ).astype(bf)

    in_maps = []
    for c in range(NCORES):
        sl = slice(c * DPC, (c + 1) * DPC)
        in_maps.append(
            {
                "xt": xt,
                "wq": wprep(Wq, sl),
                "wk": wprep(Wk, sl),
                "wv": wprep(Wv, sl),
                "wo": np.ascontiguousarray(Wo[sl, :]).astype(bf),
                "bq": np.ascontiguousarray(bq[sl]).reshape(DPC, 1).astype(np.float32),
                "bk": np.ascontiguousarray(bk[sl]).reshape(DPC, 1).astype(np.float32),
                "bv": np.ascontiguousarray(bv[sl]).reshape(1, DPC).astype(bf),
                "cos32": cos_b,
                "sin64": sin_b,
            }
        )
    return in_maps


def _get_nc():
    if "nc" not in _cache:
        _cache["nc"] = _build_nc()
    return _cache["nc"]


def run(inputs_dict, trace=False):
    """Build (cached), run on 8 cores, assemble full output. Returns
    (output fp32 [B,S,D], BassKernelResults)."""
    from concourse.bass_utils import run_bass_kernel_spmd

    nc = _get_nc()
    in_maps = _prep_inputs(
        inputs_dict["inputs"],
        inputs_dict["Wq"],
        inputs_dict["bq"],
        inputs_dict["Wk"],
        inputs_dict["bk"],
        inputs_dict["Wv"],
        inputs_dict["bv"],
        inputs_dict["Wo"],
    )
    res = run_bass_kernel_spmd(
        nc, in_maps, core_ids=list(range(NCORES)), trace=trace
    )
    acc = np.zeros((D, NS), dtype=np.float32)
    for r in res.results:
        acc += r["out"].astype(np.float32)
    out = acc.T.reshape(B, S, D) + np.asarray(inputs_dict["bo"], dtype=np.float32)
    return out.astype(np.float32), res


def kernel(**inputs):
    out, _ = run(inputs, trace=False)
    return out


# revision 24
# speedup vs baseline: 1.2021x; 1.0392x over previous
"""Trainium2 Bass kernel: 16-head attention with RoPE (dense_transformer).

Sharding: tensor-parallel over heads. 8 cores x 2 heads each.
Each core: Wq/Wk/Wv column slice [1024,128], Wo row slice [128,1024],
full input; computes its heads' attention + partial output projection.
Host sums the 8 partial outputs (row-parallel Wo reduction) and adds bo.

Device layout is "transposed": Q^T/K^T/ctx^T are kept as [dim, seq] with
the head dim on SBUF partitions, so Q^T = Wq^T @ X^T comes straight out
of the PE, scores^T = K^T.T @ Q^T needs no transposes, and the softmax
denominator falls out of an extra ones-column appended to V.

v3: fully software-pipelined emission. The softmax exp runs only on the
scalar engine (133us total) while PE streams total ~165us, so the PE
instruction stream is emitted as: [scores matmul pair for unit (b,sh,h),
tile tt] followed by ~550ns of "filler" matmuls pulled from a FIFO
(second-batch QKV projection, context matmuls of the previous unit,
output-projection tiles). This keeps the exp engine continuously fed
while the PE never head-of-line blocks on PSUM slots waiting for exp.
DMA: host-relaid-out inputs with >=2KB contiguous per-partition lines,
split across both HWDGE queues.
"""

import sys

if "/opt/trn_rl_repo" not in sys.path:
    sys.path.insert(0, "/opt/trn_rl_repo")

import collections

import numpy as np
import ml_dtypes

B = 2
S = 2048
NS = B * S  # 4096
D = 1024
H = 16
DK = 64
NCORES = 8
HPC = H // NCORES  # heads per core = 2
DPC = HPC * DK  # model dims per core = 128

_cache = {}


def _build_nc(debug_taps=False):
    import concourse.bass as bass
    import concourse.tile as tile
    import concourse.mybir as mybir
    from concourse import bacc

    fp32 = mybir.dt.float32
    bf16 = mybir.dt.bfloat16
    Exp = mybir.ActivationFunctionType.Exp

    nc = bacc.Bacc("TRN2", debug=False, num_devices=NCORES)
    dbg = {}
    if debug_taps:
        for nm, shape in (
            ("dbg_qrot", [128, NS]),
            ("dbg_krot", [128, NS]),
            ("dbg_v", [128, 32 * 2 * (DK + 1)]),
            ("dbg_ctxT", [128, NS]),
        ):
            dbg[nm] = nc.dram_tensor(nm, shape, bf16, kind="ExternalOutput").ap()

    # xt: [p, st, ch, j] = X[st*512+j, ch*128+p]; per-partition line for a
    # given st is 8KB contiguous.
    xt = nc.dram_tensor("xt", [128, 8, 8, 512], bf16, kind="ExternalInput").ap()
    wq = nc.dram_tensor("wq", [128, 8, 128], bf16, kind="ExternalInput").ap()
    wk = nc.dram_tensor("wk", [128, 8, 128], bf16, kind="ExternalInput").ap()
    wv = nc.dram_tensor("wv", [128, 8, 128], bf16, kind="ExternalInput").ap()
    wo = nc.dram_tensor("wo", [DPC, D], bf16, kind="ExternalInput").ap()
    bq = nc.dram_tensor("bq", [DPC, 1], fp32, kind="ExternalInput").ap()
    bk = nc.dram_tensor("bk", [DPC, 1], fp32, kind="ExternalInput").ap()
    bv = nc.dram_tensor("bv", [1, DPC], bf16, kind="ExternalInput").ap()
    cos_d = nc.dram_tensor("cos32", [32, S], bf16, kind="ExternalInput").ap()
    sin_d = nc.dram_tensor("sin64", [64, S], bf16, kind="ExternalInput").ap()
    out_d = nc.dram_tensor("out", [D, NS], bf16, kind="ExternalOutput").ap()

    with tile.TileContext(nc) as tc:
        with (
            tc.tile_pool(name="persist", bufs=1) as persist,
            tc.tile_pool(name="ropetmp", bufs=1) as ropetmp,
            tc.tile_pool(name="att_sb", bufs=2) as att_sb,
            tc.tile_pool(name="op_sb", bufs=4) as op_sb,
            tc.tile_pool(name="den_p", bufs=1) as den_p,
            tc.tile_pool(name="r_sb", bufs=1) as r_sb,
            tc.tile_pool(name="r_dram", bufs=2, space="DRAM") as r_dram,
            tc.tile_pool(name="sc_ps", bufs=2, space="PSUM") as sc_ps,
            tc.tile_pool(name="ctx_ps", bufs=2, space="PSUM") as ctx_ps,
            tc.tile_pool(name="mix_ps", bufs=2, space="PSUM") as mix_ps,
        ):
            qrot = persist.tile([128, NS], bf16, tag="qrot")
            krot = persist.tile([128, NS], bf16, tag="krot")
            v_sb = persist.tile([128, 32, 2 * (DK + 1)], bf16, tag="v")
            ctxT = persist.tile([128, NS], bf16, tag="ctxT")
            wo_sb = persist.tile([128, 8, 128], bf16, tag="wo")
            cos_sb = persist.tile([128, S], bf16, tag="cos")
            sin_sb = persist.tile([128, S], bf16, tag="sin")
            xt_sb = persist.tile([128, 8, 8, 512], bf16, tag="xt")
            wq_sb = persist.tile([128, 8, 128], bf16, tag="wq")
            wk_sb = persist.tile([128, 8, 128], bf16, tag="wk")
            wv_sb = persist.tile([128, 8, 128], bf16, tag="wv")
            bq_sb = persist.tile([128, 1], fp32, tag="bq")
            bk_sb = persist.tile([128, 1], fp32, tag="bk")
            bvb = persist.tile([128, 128], bf16, tag="bvb")
            dn128 = den_p.tile([128, 64], fp32, tag="dn128", bufs=1)
            recip128 = den_p.tile([128, 64], bf16, tag="recip", bufs=1)

            # ---- input DMA schedule ----
            nc.sync.dma_start(wq_sb[:], wq)
            nc.scalar.dma_start(wk_sb[:], wk)
            nc.scalar.dma_start(wv_sb[:], wv)
            for st in (0, 2, 4, 6):
                nc.sync.dma_start(xt_sb[:, st], xt[:, st])
            nc.scalar.dma_start(cos_sb[0:32, :], cos_d)
            nc.scalar.dma_start(sin_sb[0:64, :], sin_d)
            nc.gpsimd.dma_start(cos_sb[32:64, :], cos_sb[0:32, :])
            nc.gpsimd.dma_start(cos_sb[64:128, :], cos_sb[0:64, :])
            nc.gpsimd.dma_start(sin_sb[64:128, :], sin_sb[0:64, :])
            nc.gpsimd.dma_start(bq_sb[:], bq)
            nc.gpsimd.dma_start(bk_sb[:], bk)
            nc.gpsimd.dma_start(bvb[:], bv.to_broadcast((128, 128)))
            nc.scalar.dma_start(xt_sb[:, 1], xt[:, 1])
            nc.scalar.dma_start(xt_sb[:, 3], xt[:, 3])
            nc.sync.dma_start(wo_sb[:], wo.rearrange("p (j m) -> p j m", m=128))

            # ---------------- emission helpers ----------------
            def qk_group(w_sb_, b_sb_, st, rot, drain_eng):
                def fn():
                    ps = mix_ps.tile([128, 512], fp32, tag="mix")
                    for ch in range(8):
                        nc.tensor.matmul(
                            ps[:], w_sb_[:, ch, :], xt_sb[:, st, ch, :],
                            start=(ch == 0), stop=(ch == 7),
                        )
                    sl = slice(st * 512, (st + 1) * 512)
                    if drain_eng == "scalar":
                        nc.scalar.add(rot[:, sl], ps[:], add=b_sb_[:])
                    else:
                        nc.vector.tensor_scalar_add(rot[:, sl], ps[:], b_sb_[:])
                return fn

            def v_group(tt, drain_eng="vector"):
                def fn():
                    st, tv = tt // 4, tt % 4
                    ps = mix_ps.tile([128, 512], fp32, tag="mix")
                    psv = ps[:, 0:128]
                    for ch in range(8):
                        nc.tensor.matmul(
                            psv,
                            xt_sb[:, st, ch, tv * 128 : (tv + 1) * 128],
                            wv_sb[:, ch, :],
                            start=(ch == 0), stop=(ch == 7),
                        )
                    dst = v_sb[:, tt].rearrange("p (h x) -> p h x", h=2)[:, :, 0:DK]
                    if drain_eng == "scalar":
                        nc.scalar.add(dst, psv, add=0.0)
                        nc.gpsimd.tensor_add(dst, dst, bvb[:])
                    else:
                        nc.vector.tensor_add(dst, psv, bvb[:])
                return fn

            def rope_st(st):
                def fn():
                    csl = slice(st * 512, (st + 1) * 512)
                    cs = slice((st % 4) * 512, (st % 4 + 1) * 512)
                    for rot in (krot, qrot):
                        swap = ropetmp.tile(
                            [128, 512], bf16, tag="swap", bufs=2, name="swap"
                        )
                        for g in (0, 64):
                            nc.gpsimd.dma_start(
                                swap[g : g + 32, :], rot[g + 32 : g + 64, csl]
                            )
                            nc.gpsimd.dma_start(
                                swap[g + 32 : g + 64, :], rot[g : g + 32, csl]
                            )
                        nc.vector.tensor_mul(swap[:], swap[:], sin_sb[:, cs])
                        nc.vector.tensor_mul(rot[:, csl], rot[:, csl], cos_sb[:, cs])
                        nc.vector.tensor_add(rot[:, csl], rot[:, csl], swap[:])
                return fn

            # ctx state: pc tiles keyed by (b, sh, h, sq); den rows per (b, sh)
            pc_tiles = {}
            den_rows = {}

            def ctx_chunk(b, sh, h, sq, t0, expS):
                def fn():
                    key = (b, sh, h, sq)
                    if t0 == 0:
                        pc_tiles[key] = ctx_ps.tile([DK + 1, 512], fp32, tag="pc", name="pc")
                    pc = pc_tiles[key]
                    for tt in range(t0, t0 + 4):
                        nc.tensor.matmul(
                            pc[:],
                            v_sb[:, b * 16 + tt, h * (DK + 1) : (h + 1) * (DK + 1)],
                            expS[:, tt, sq * 512 : (sq + 1) * 512],
                            start=(tt == 0), stop=(tt == 15),
                        )
                    if t0 == 12:
                        # drain: ctx rows -> ctxT (pre-division), den row
                        st_i = sh * 2 + sq
                        ds0 = b * S + st_i * 512
                        if (b, sh) not in den_rows:
                            den_rows[(b, sh)] = den_p.tile(
                                [1, 2048], fp32, tag="den", name="den_row"
                            )
                        doff = (h * 2 + sq) * 512
                        nc.vector.tensor_copy(
                            den_rows[(b, sh)][:, doff : doff + 512],
                            pc[DK : DK + 1, :],
                        )
                        if h == 0:
                            nc.vector.tensor_copy(
                                ctxT[0:DK, ds0 : ds0 + 512], pc[0:DK, :]
                            )
                        else:
                            stg = r_sb.tile([64, 512], bf16, tag="stg", bufs=1, name="stg")
                            nc.vector.tensor_copy(stg[:], pc[0:DK, :])
                            nc.vector.stream_shuffle(
                                ctxT[DK : 2 * DK, ds0 : ds0 + 512],
                                stg[:],
                                mask=list(range(32)),
                            )
                        del pc_tiles[key]
                return fn

            def den_div(b, sh):
                def fn():
                    # denominators for this (b, sh): 4 rows of 512
                    coff = (b * 2 + sh) * 16
                    dd = r_dram.tile([2048], fp32, tag="den_dram")
                    rd = r_dram.tile([2048], bf16, tag="rec_dram")
                    nc.sync.dma_start(
                        dd.rearrange("(o c) -> o c", o=1),
                        den_rows.pop((b, sh))[0:1, :],
                    )
                    nc.sync.dma_start(
                        dn128[:, coff : coff + 16],
                        dd.rearrange("(p c) -> p c", p=128),
                    )
                    with nc.allow_low_precision(
                        reason="bf16 softmax reciprocal is within kernel tolerance"
                    ):
                        nc.vector.reciprocal(
                            recip128[:, coff : coff + 16],
                            dn128[:, coff : coff + 16],
                        )
                    nc.sync.dma_start(
                        rd.rearrange("(p c) -> p c", p=128),
                        recip128[:, coff : coff + 16],
                    )
                    Rt = r_sb.tile([128, 1024], bf16, tag="R")
                    for h in range(HPC):
                        nc.sync.dma_start(
                            Rt[h * DK : (h + 1) * DK, :],
                            rd[h * 1024 : (h + 1) * 1024]
                            .rearrange("(o s) -> o s", o=1)
                            .to_broadcast((DK, 1024)),
                        )
                    c0 = b * S + sh * 1024
                    nc.vector.tensor_mul(
                        ctxT[:, c0 : c0 + 1024], ctxT[:, c0 : c0 + 1024], Rt[:]
                    )
                return fn

            def op_tile(b, stp, oc, tail=False):
                def fn():
                    c0 = b * S + stp * 1024
                    ob2 = op_sb.tile([128, 1024], bf16, tag="ob2")
                    if tail and oc % 2 == 0:
                        ps2 = sc_ps.tile([128, 1024], fp32, tag="ps", name="ps2")
                        pa, pb = ps2[:, 0:512], ps2[:, 512:1024]
                    else:
                        pa = mix_ps.tile([128, 512], fp32, tag="mix", name="pa")
                        pb = mix_ps.tile([128, 512], fp32, tag="mix", name="pb")
                    nc.tensor.matmul(
                        pa, wo_sb[:, oc, :], ctxT[:, c0 : c0 + 512],
                        start=True, stop=True,
                    )
                    nc.tensor.matmul(
                        pb, wo_sb[:, oc, :], ctxT[:, c0 + 512 : c0 + 1024],
                        start=True, stop=True,
                    )
                    nc.vector.tensor_copy(ob2[:, 0:512], pa)
                    nc.scalar.copy(ob2[:, 512:1024], pb)
                    nc.sync.dma_start(
                        out_d[oc * 128 : (oc + 1) * 128, c0 : c0 + 1024], ob2[:]
                    )
                return fn

            # ---------------- filler queue + debt-carry metering ------------
            fq_hi = collections.deque()
            fq_lo = fq_hi  # single FIFO: emission order must respect data deps
            acc = [0.0]

            def pull(budget):
                acc[0] += budget
                while acc[0] > 0 and fq_hi:
                    cost, fn = fq_hi.popleft()
                    fn()
                    acc[0] -= cost

            # upfront (unmetered, overlaps the input DMA): st0/st1 K/Q/rope
            # plus V(b0) for those chunks slotted into the DMA-wait gaps
            for st in (0, 1):
                qk_group(wk_sb, bk_sb, st, krot, "vector")()
                qk_group(wq_sb, bq_sb, st, qrot, "vector")()
                rope_st(st)()
                for tv in range(4):
                    v_group(st * 4 + tv)()
            nc.vector.memset(
                v_sb[:].rearrange("p t (h x) -> p t h x", x=DK + 1)[:, 0:16, :, DK],
                1.0,
            )

            # seed fillers: st2/st3 K/Q/rope + rest of V(b0)
            for st in (2, 3):
                fq_lo.append((2000, qk_group(wk_sb, bk_sb, st, krot, "vector")))
                fq_lo.append((2000, qk_group(wq_sb, bq_sb, st, qrot, "vector")))
                fq_lo.append((0, rope_st(st)))

            def _xt_b1():
                nc.gpsimd.dma_start(xt_sb[:, 5], xt[:, 5])
                nc.gpsimd.dma_start(xt_sb[:, 7], xt[:, 7])

            fq_lo.append((0, _xt_b1))
            for tt in range(8, 16):
                fq_lo.append((700, v_group(tt)))

            units = [
                (b, sh, h) for b in range(B) for sh in range(2) for h in range(HPC)
            ]
            for j, (b, sh, h) in enumerate(units):
                hh = h * DK
                expS = att_sb.tile([128, 16, 1024], bf16, tag="expS")
                for tt in range(16):
                    ps = sc_ps.tile([128, 1024], fp32, tag="ps")
                    for si in range(2):
                        s0 = b * S + sh * 1024 + si * 512
                        nc.tensor.matmul(
                            ps[:, si * 512 : (si + 1) * 512],
                            krot[
                                hh : hh + DK,
                                b * S + tt * 128 : b * S + (tt + 1) * 128,
                            ],
                            qrot[hh : hh + DK, s0 : s0 + 512],
                            start=True, stop=True,
                        )
                    nc.scalar.activation(expS[:, tt, :], ps[:], Exp, scale=0.125)
                    pull(960)

                # enqueue this unit's ctx work (latency-sensitive)
                for sq in range(2):
                    for t0 in (0, 4, 8, 12):
                        fq_hi.append((1100, ctx_chunk(b, sh, h, sq, t0, expS)))
                if h == 1:
                    fq_hi.append((0, den_div(b, sh)))
                    tail = j == len(units) - 1
                    for oc in range(8):
                        fq_hi.append((700, op_tile(b, sh, oc, tail=tail)))
                # strategic phase-1(b1) injection
                if j == 0:
                    for st in range(4, 8):
                        fq_lo.append((2000, qk_group(wk_sb, bk_sb, st, krot, "vector")))
                        fq_lo.append((2000, qk_group(wq_sb, bq_sb, st, qrot, "vector")))
                        fq_lo.append((0, rope_st(st)))
                if j == 1:
                    for tt in range(16, 32):
                        fq_lo.append((700, v_group(tt)))
                    fq_lo.append(
                        (
                            200,
                            lambda: nc.vector.memset(
                                v_sb[:].rearrange(
                                    "p t (h x) -> p t h x", x=DK + 1
                                )[:, 16:32, :, DK],
                                1.0,
                            ),
                        )
                    )

            pull(float("inf"))

            if debug_taps:
                nc.sync.dma_start(dbg["dbg_qrot"], qrot[:])
                nc.sync.dma_start(dbg["dbg_krot"], krot[:])
                nc.sync.dma_start(dbg["dbg_v"], v_sb[:].rearrange("p a b -> p (a b)"))
                nc.sync.dma_start(dbg["dbg_ctxT"], ctxT[:])

    nc.compile()
    return nc


def _rope_tables():
    pos = np.arange(S, dtype=np.float64)
    inv_freq = np.exp(np.arange(0, DK, 2, dtype=np.float64) * (-np.log(10000.0) / DK))
    ang = pos[:, None] * inv_freq[None, :]  # [S, 32]
    c = np.cos(ang).astype(np.float32).T  # [32, S]
    s = np.sin(ang).astype(np.float32).T
    cos32 = c
    sin64 = np.concatenate([-s, s], axis=0)  # [64, S]
    return cos32, sin64


def _prep_inputs(inputs, Wq, bq, Wk, bk, Wv, bv, Wo):
    bf = ml_dtypes.bfloat16
    x2 = np.asarray(inputs, dtype=np.float32).reshape(NS, D)
    # [p, st, ch, j] = X[st*512+j, ch*128+p]
    xt = np.ascontiguousarray(
        x2.reshape(8, 512, 8, 128).transpose(3, 0, 2, 1)
    ).astype(bf)
    cos32, sin64 = _rope_tables()
    cos_b = cos32.astype(bf)
    sin_b = sin64.astype(bf)

    def wprep(W, sl):
        return np.ascontiguousarray(
            W[:, sl].reshape(8, 128, DPC).transpose(1, 0, 2)
        ).astype(bf)

    in_maps = []
    for c in range(NCORES):
        sl = slice(c * DPC, (c + 1) * DPC)
        in_maps.append(
            {
                "xt": xt,
                "wq": wprep(Wq, sl),
                "wk": wprep(Wk, sl),
                "wv": wprep(Wv, sl),
                "wo": np.ascontiguousarray(Wo[sl, :]).astype(bf),
                "bq": np.ascontiguousarray(bq[sl]).reshape(DPC, 1).astype(np.float32),
                "bk": np.ascontiguousarray(bk[sl]).reshape(DPC, 1).astype(np.float32),
                "bv": np.ascontiguousarray(bv[sl]).reshape(1, DPC).astype(bf),
                "cos32": cos_b,
                "sin64": sin_b,
            }
        )
    return in_maps


def _get_nc():
    if "nc" not in _cache:
        _cache["nc"] = _build_nc()
    return _cache["nc"]


def run(inputs_dict, trace=False):
    """Build (cached), run on 8 cores, assemble full output. Returns
    (output fp32 [B,S,D], BassKernelResults)."""
    from concourse.bass_utils import run_bass_kernel_spmd

    nc = _get_nc()
    in_maps = _prep_inputs(
        inputs_dict["inputs"],
        inputs_dict["Wq"],
        inputs_dict["bq"],
        inputs_dict["Wk"],
        inputs_dict["bk"],
        inputs_dict["Wv"],
        inputs_dict["bv"],
        inputs_dict["Wo"],
    )
    res = run_bass_kernel_spmd(
        nc, in_maps, core_ids=list(range(NCORES)), trace=trace
    )
    acc = np.zeros((D, NS), dtype=np.float32)
    for r in res.results:
        acc += r["out"].astype(np.float32)
    out = acc.T.reshape(B, S, D) + np.asarray(inputs_dict["bo"], dtype=np.float32)
    return out.astype(np.float32), res


def kernel(**inputs):
    out, _ = run(inputs, trace=False)
    return out
